# revision 45
# baseline (speedup 1.0000x reference)
"""ComGNNBank Trainium2 kernel: 2-layer community-GIN bank + input encoder.

Strategy (8 NeuronCores, SPMD):
  - Nodes sharded by dst: core c owns rows [c*6250, (c+1)*6250).
  - Edges bucketed by dst into 128-node buckets; per bucket a weighted
    scatter-add is done on the tensor engine: one-hot(dst) matmul with
    PSUM accumulation over 128-edge blocks.
  - Gather of source-node features via SWDGE dma_gather (4 SWDGE queues,
    greedy load-balanced; one call per (2-bucket group, table half))
    from bf16 replicas of the feature table (x / h1). The replica is
    split in two halves (buckets 0..SPLIT_BKT-1 / rest) AllGather-ed
    separately so the first half's transfer overlaps producer compute
    and the first gathers overlap the second transfer.
  - GIN MLP runs feature-major (transposed, bf16) so BatchNorm+ReLU
    fuse into one per-partition scalar-engine op.
  - BN statistics: per-group partial sums inline, tiny AllReduce.
"""

import math

import ml_dtypes
import numpy as np

import concourse.bacc as bacc
import concourse.bass as bass
import concourse.mybir as mybir
import concourse.tile as tile
from concourse.bass_utils import run_bass_kernel_spmd

F32 = mybir.dt.float32
BF16 = mybir.dt.bfloat16
I16 = mybir.dt.int16
AF = mybir.ActivationFunctionType
OP = mybir.AluOpType
AX = mybir.AxisListType

P = 128
D = 256            # feature dim (in and emb)
NCOM = 4
DCOM = 64
N_CORES = 8
BN_EPS = 1e-5
IDX_LIMIT = 32768  # int16 gather index limit
SPLIT_BKT = 30     # buckets 0..29 -> table A (AllGather chunk 1)
PRO = 3            # A-gather prefetch depth (groups)
N_SWDGE_Q = 4


class _Cfg:
    def __init__(self, n_nodes):
        assert n_nodes % N_CORES == 0
        self.n_nodes = n_nodes
        self.npc = n_nodes // N_CORES
        self.nbkt = math.ceil(self.npc / P)
        self.ncols = self.nbkt * P          # padded per-core node slots
        self.last_rows = self.npc - (self.nbkt - 1) * P
        self.rows_a = min(SPLIT_BKT * P, self.npc)
        self.rows_b = self.npc - self.rows_a
        self.tbl_a = N_CORES * self.rows_a
        self.tbl_b = N_CORES * self.rows_b
        assert self.tbl_a < IDX_LIMIT and self.tbl_b < IDX_LIMIT

CFG = _Cfg(50000)


# ----------------------------------------------------------------------------
# Host planner
# ----------------------------------------------------------------------------

def blk_layout(nb_seg, nbkt):
    """Block-column layout: per 2-bucket group g the blocks are laid out
    [A(b0) | A(b1) | B(b0) | B(b1)] so one gather call covers each
    (group, half). Returns per-bucket segments, per-group call ranges,
    and the total block count."""
    blk = 0
    bucket_segs = [[] for _ in range(nbkt)]
    call_segs = []
    for g0 in range(0, nbkt, 2):
        bs = [b for b in (g0, g0 + 1) if b < nbkt]
        entry = []
        for h in range(2):
            col0 = blk
            for b in bs:
                nb = int(nb_seg[b, h])
                bucket_segs[b].append((blk, nb))
                blk += nb
            entry.append((col0, blk - col0))
        call_segs.append(entry)
    return bucket_segs, call_segs, blk


def _wrap_idx(seg_idx):
    """int16 index list (len%128==0) -> [128, len//16] wrapped layout."""
    w = seg_idx.reshape(-1, 16).T.astype(np.int16)  # [16, n//16]
    return np.tile(w, (8, 1))


def plan(edge_index, ew, cfg=CFG):
    """Bucket/pad edges; returns per-core input arrays + shared structure."""
    src = np.asarray(edge_index[0], np.int64)
    dst = np.asarray(edge_index[1], np.int64)
    ew = np.asarray(ew, np.float32)
    core = dst // cfg.npc
    ldst = dst - core * cfg.npc
    bkt = ldst // P
    off = ldst % P
    sc = src // cfg.npc
    sr = src - sc * cfg.npc
    half = (sr >= cfg.rows_a).astype(np.int64)
    ridx = np.where(half == 0, sc * cfg.rows_a + sr,
                    sc * cfg.rows_b + (sr - cfg.rows_a))

    # group edges by (core, bucket, half)
    key = (core * cfg.nbkt + bkt) * 2 + half
    order = np.argsort(key, kind="stable")
    key_s = key[order]
    nkeys = N_CORES * cfg.nbkt * 2
    starts = np.searchsorted(key_s, np.arange(nkeys + 1))
    counts = np.diff(starts).reshape(N_CORES, cfg.nbkt, 2)
    nb_seg = np.ceil(counts / P).astype(np.int64).max(axis=0)  # [nbkt, 2]

    bucket_segs, call_segs, nb_tot = blk_layout(nb_seg, cfg.nbkt)

    ridx_s = ridx[order]
    off_s = off[order]
    ew_s = ew[:, order]

    idx_ts, dst_ts, w4_ts = [], [], []
    for c in range(N_CORES):
        idx_all = np.zeros(nb_tot * P, np.int64)
        dst_all = np.full(nb_tot * P, -1.0, np.float32)
        w4_all = np.zeros((nb_tot * P, NCOM), np.float32)
        for b in range(cfg.nbkt):
            for h in range(2):
                col, nb = bucket_segs[b][h]
                if nb == 0:
                    continue
                g = (c * cfg.nbkt + b) * 2 + h
                s0, s1 = starts[g], starts[g + 1]
                n = s1 - s0
                a0 = col * P
                idx_all[a0:a0 + n] = ridx_s[s0:s1]
                dst_all[a0:a0 + n] = off_s[s0:s1]
                w4_all[a0:a0 + n] = ew_s[:, s0:s1].T
        idx_t = np.zeros((P, nb_tot * 8), np.int16)
        for b in range(cfg.nbkt):
            for h in range(2):
                col, nb = bucket_segs[b][h]
                if nb == 0:
                    continue
                seg = idx_all[col * P:(col + nb) * P]
                idx_t[:, col * 8:(col + nb) * 8] = _wrap_idx(seg)
        dst_t = np.repeat(dst_all.reshape(nb_tot, P).T, 2,
                          axis=1).astype(ml_dtypes.bfloat16)
        w4_t = (
            np.repeat(w4_all, 2, axis=1)
            .reshape(nb_tot, P, 2 * NCOM)
            .transpose(1, 0, 2)
            .reshape(P, nb_tot * 2 * NCOM)
            .astype(ml_dtypes.bfloat16)
        )
        idx_ts.append(idx_t)
        dst_ts.append(dst_t)
        w4_ts.append(w4_t)

    meta = {"nb_seg": nb_seg, "nb_tot": nb_tot}
    return meta, idx_ts, dst_ts, w4_ts


def prep_weights(inp, cfg=CFG):
    """Shared (replicated) parameter tensors in kernel layout."""
    def cat_w1(w):  # [4, ci, 64] -> [ci_tot=256 blockwise, 256]
        ci = w.shape[1]
        if ci == D:  # layer-1 first mm: dense columns concat
            return np.concatenate([w[k] for k in range(NCOM)], axis=1)
        out = np.zeros((D, D), np.float32)  # block-diag for 64-in mats
        for k in range(NCOM):
            out[k * DCOM:(k + 1) * DCOM, k * DCOM:(k + 1) * DCOM] = w[k]
        return out

    def reorder(w):  # [256, 256] -> [128, 512]: [c%128, (c//128)*256 + f]
        return np.concatenate([w[0:P, :], w[P:2 * P, :]],
                              axis=1).astype(ml_dtypes.bfloat16)

    def halves(v):  # [256] -> [128, 2]
        return np.stack([v[0:P], v[P:2 * P]], axis=1).astype(np.float32)

    w1l1 = cat_w1(np.asarray(inp["W1_0"], np.float32))
    w2l1 = cat_w1(np.asarray(inp["W2_0"], np.float32))
    w1l2 = cat_w1(np.asarray(inp["W1_1"], np.float32))
    w2l2 = cat_w1(np.asarray(inp["W2_1"], np.float32))

    out = {
        "W1r_l1": reorder(w1l1),
        "W2r_l1": reorder(w2l1),
        "W1r_l2": reorder(w1l2),
        "W2r_l2": reorder(w2l2),
        "encWr": reorder(np.asarray(inp["encW"], np.float32)),
        "b1c_l1": halves(np.asarray(inp["b1_0"], np.float32).reshape(-1)),
        "b2c_l1": halves(np.asarray(inp["b2_0"], np.float32).reshape(-1)),
        "b1c_l2": halves(np.asarray(inp["b1_1"], np.float32).reshape(-1)),
        "b2c_l2": halves(np.asarray(inp["b2_1"], np.float32).reshape(-1)),
        "gm1": halves(np.asarray(inp["g0"], np.float32).reshape(-1)),
        "btc1": halves(np.asarray(inp["bt0"], np.float32).reshape(-1)),
        "gm2": halves(np.asarray(inp["g1"], np.float32).reshape(-1)),
        "btc2": halves(np.asarray(inp["bt1"], np.float32).reshape(-1)),
        "encb_row": np.asarray(inp["encb"], np.float32).reshape(1, D)
        .astype(ml_dtypes.bfloat16),
        "ones1": np.ones((1, P), ml_dtypes.bfloat16),
        "iota16": np.broadcast_to(
            np.arange(P, dtype=np.float32), (P, P)
        ).astype(ml_dtypes.bfloat16).copy(),
        "ident": np.eye(P, dtype=np.float32),
    }
    return out


# ----------------------------------------------------------------------------
# Kernel builder
# ----------------------------------------------------------------------------

def _ap3(t_ap, ap_list):
    """Rebuild an AP with an explicit free-dim pattern list."""
    return bass.AP(t_ap.tensor, t_ap.offset, [list(t_ap.ap[0])] + ap_list)


def build_nc(meta, cfg=CFG):
    nb_seg = meta["nb_seg"]
    nb_tot = meta["nb_tot"]
    nbkt = cfg.nbkt
    ncols = cfg.ncols
    bucket_segs, call_segs, nb_tot2 = blk_layout(nb_seg, nbkt)
    assert nb_tot2 == nb_tot
    ngrp = len(call_segs)
    nbAmax = max(e[0][1] for e in call_segs)
    nbBmax = max(e[1][1] for e in call_segs)
    rg = [list(range(N_CORES))]

    nc = bacc.Bacc("TRN2", target_bir_lowering=False, debug=False,
                   num_devices=N_CORES, num_swdge_queues=N_SWDGE_Q)
    qload = [0] * N_SWDGE_Q

    def next_q(nblocks):
        q = qload.index(min(qload))
        qload[q] += nblocks
        return q

    # inputs
    x_self = nc.dram_tensor("x_self", [cfg.npc, D], F32, kind="ExternalInput")
    idx_d = nc.dram_tensor("idx_t", [P, nb_tot * 8], I16, kind="ExternalInput")
    dst_d = nc.dram_tensor("dst_t", [P, nb_tot * 2], BF16,
                           kind="ExternalInput")
    w4_d = nc.dram_tensor("w4_t", [P, nb_tot * 2 * NCOM], BF16,
                          kind="ExternalInput")
    wd = {}
    for nm, shape, dt in [
        ("W1r_l1", [P, 2 * D], BF16), ("W2r_l1", [P, 2 * D], BF16),
        ("W1r_l2", [P, 2 * D], BF16), ("W2r_l2", [P, 2 * D], BF16),
        ("encWr", [P, 2 * D], BF16),
        ("b1c_l1", [P, 2], F32), ("b2c_l1", [P, 2], F32),
        ("b1c_l2", [P, 2], F32), ("b2c_l2", [P, 2], F32),
        ("gm1", [P, 2], F32), ("btc1", [P, 2], F32),
        ("gm2", [P, 2], F32), ("btc2", [P, 2], F32),
        ("encb_row", [1, D], BF16), ("ones1", [1, P], BF16),
        ("iota16", [P, P], BF16), ("ident", [P, P], F32),
    ]:
        wd[nm] = nc.dram_tensor(nm, shape, dt, kind="ExternalInput")

    # outputs
    enc_o = nc.dram_tensor("enc_o", [cfg.npc, D], F32, kind="ExternalOutput")
    out1_o = nc.dram_tensor("out1_o", [cfg.npc, D], F32, kind="ExternalOutput")
    out2_o = nc.dram_tensor("out2_o", [cfg.npc, D], F32, kind="ExternalOutput")

    with tile.TileContext(nc) as tc:
        with (
            tc.tile_pool(name="const", bufs=1) as cpool,
            tc.tile_pool(name="big", bufs=1) as bigpool,
            tc.tile_pool(name="gatA", bufs=PRO + 2) as gpoolA,
            tc.tile_pool(name="gatB", bufs=3) as gpoolB,
            tc.tile_pool(name="work", bufs=3) as wpool,
            tc.tile_pool(name="rows", bufs=3) as rpool,
            tc.tile_pool(name="stats", bufs=1) as spool,
            tc.tile_pool(name="psA", bufs=2, space="PSUM") as psA,
            tc.tile_pool(name="psT", bufs=2, space="PSUM") as psT,
            tc.tile_pool(name="ps1", bufs=2, space="PSUM") as ps1,
            tc.tile_pool(name="ps2", bufs=2, space="PSUM") as ps2,
            tc.tile_pool(name="dram", bufs=1, space="DRAM") as dpool,
        ):
            # ---- constants into SBUF
            sb = {}
            for nm in wd:
                t = cpool.tile(list(wd[nm].shape), wd[nm].dtype, tag=nm)
                nc.sync.dma_start(t[:], wd[nm].ap())
                sb[nm] = t
            idx_sb = cpool.tile([P, nb_tot * 8], I16, tag="idx")
            nc.sync.dma_start(idx_sb[:], idx_d.ap())
            dst_sb = cpool.tile([P, nb_tot * 2], BF16, tag="dst")
            nc.sync.dma_start(dst_sb[:], dst_d.ap())
            w4_sb = cpool.tile([P, nb_tot * 2 * NCOM], BF16, tag="w4")
            nc.sync.dma_start(w4_sb[:], w4_d.ap())

            # ---- DRAM scratch
            yslA = dpool.tile([cfg.rows_a, D], BF16, tag="yslA")
            yslB = dpool.tile([cfg.rows_b, D], BF16, tag="yslB")
            yfA = dpool.tile([cfg.tbl_a, D], BF16, tag="yfA",
                             addr_space="Shared")
            yfB = dpool.tile([cfg.tbl_b, D], BF16, tag="yfB",
                             addr_space="Shared")
            y_self = dpool.tile([cfg.npc, D], F32, tag="yself")
            hslA = dpool.tile([cfg.rows_a, D], BF16, tag="hslA")
            hslB = dpool.tile([cfg.rows_b, D], BF16, tag="hslB")
            hfA = dpool.tile([cfg.tbl_a, D], BF16, tag="hfA",
                             addr_space="Shared")
            hfB = dpool.tile([cfg.tbl_b, D], BF16, tag="hfB",
                             addr_space="Shared")
            h1_self = dpool.tile([cfg.npc, D], F32, tag="h1self")
            st_in1 = dpool.tile([P, 4], F32, tag="sti1")
            st_out1 = dpool.tile([P, 4], F32, tag="sto1", addr_space="Shared")
            st_in2 = dpool.tile([P, 4], F32, tag="sti2")
            st_out2 = dpool.tile([P, 4], F32, tag="sto2", addr_space="Shared")

            def bucket_rows(b):
                return cfg.last_rows if b == nbkt - 1 else P

            def groups():
                for gi, g0 in enumerate(range(0, nbkt, 2)):
                    yield gi, [b for b in (g0, g0 + 1) if b < nbkt]

            def pair_dram(dram_slice, nb_b):
                """AP writing [P, nb_b, D] sbuf tile to nb_b*P dram rows."""
                return _ap3(dram_slice, [[P * D, nb_b], [1, D]])

            def load_pair_rows(src_dram, g0, bs, tag):
                """One DMA for a group's self rows; zero-pads last bucket."""
                t = rpool.tile([P, 2, D], F32, tag=tag)
                last = bs[-1] == nbkt - 1
                rows_last = bucket_rows(bs[-1])
                if last:
                    if len(bs) == 2:
                        nc.sync.dma_start(
                            t[:, 0, :], src_dram[g0 * P:(g0 + 1) * P, :])
                        nc.vector.memset(
                            t[(rows_last // 32) * 32:P, 1, :], 0.0)
                        nc.sync.dma_start(
                            t[0:rows_last, 1, :],
                            src_dram[(g0 + 1) * P:(g0 + 1) * P + rows_last, :])
                    else:
                        nc.vector.memset(
                            t[(rows_last // 32) * 32:P, 0, :], 0.0)
                        nc.sync.dma_start(
                            t[0:rows_last, 0, :],
                            src_dram[g0 * P:g0 * P + rows_last, :])
                else:
                    nc.sync.dma_start(
                        t[:, 0:len(bs), :],
                        pair_dram(src_dram[g0 * P:g0 * P + P, :], len(bs)))
                return t

            def store_pair(dram, g0, bs, t, fh_dim=False):
                """Store [P, len(bs), D] tile to dram rows g0*P..; handles
                the short last bucket."""
                nb_b = len(bs)
                rows_last = bucket_rows(bs[-1])
                if rows_last == P:
                    nc.sync.dma_start(
                        pair_dram(dram[g0 * P:g0 * P + P, :], nb_b),
                        t[:, 0:nb_b, :])
                else:
                    for bi, b in enumerate(bs):
                        rows = bucket_rows(b)
                        nc.sync.dma_start(
                            dram[b * P:b * P + rows, :], t[0:rows, bi, :])

            # ---- phase 0: per bucket compute enc = x@encW + encb and
            #      Y = x@W1cat; export Y f32 (self) + bf16 (gather halves).
            for gi, bs in groups():
                g0 = bs[0]
                xr2 = load_pair_rows(x_self.ap(), g0, bs, "xrows")
                es2 = rpool.tile([P, 2, D], F32, tag="st32")
                ys2 = rpool.tile([P, 2, D], F32, tag="st32")
                y162 = rpool.tile([P, 2, D], BF16, tag="st16")
                for bi, b in enumerate(bs):
                    trp = psT.tile([P, D], F32, tag="tr")
                    for ch in range(2):
                        nc.tensor.transpose(
                            trp[:, ch * P:(ch + 1) * P],
                            xr2[:, bi, ch * P:(ch + 1) * P], sb["ident"][:])
                    xT = wpool.tile([P, D], BF16, tag="encxT")
                    for ch in range(2):
                        nc.scalar.copy(xT[:, ch * P:(ch + 1) * P],
                                       trp[:, ch * P:(ch + 1) * P])
                    ep = psA.tile([P, D], F32, tag="aggps")
                    for ch in range(2):
                        nc.tensor.matmul(
                            ep[:], xT[:, ch * P:(ch + 1) * P],
                            sb["encWr"][:, ch * D:(ch + 1) * D],
                            start=(ch == 0), stop=False)
                    nc.tensor.matmul(
                        ep[:], sb["ones1"][:], sb["encb_row"][:],
                        start=False, stop=True)
                    nc.vector.tensor_copy(es2[:, bi, :], ep[:])
                    yp = ps1.tile([P, 2 * D], F32, tag="z1ps")
                    for ch in range(2):
                        nc.tensor.matmul(
                            yp[:, 0:D], xT[:, ch * P:(ch + 1) * P],
                            sb["W1r_l1"][:, ch * D:(ch + 1) * D],
                            start=(ch == 0), stop=(ch == 1))
                    nc.vector.tensor_copy(ys2[:, bi, :], yp[:, 0:D])
                    nc.scalar.copy(y162[:, bi, :], yp[:, 0:D])
                store_pair(enc_o, g0, bs, es2)
                store_pair(y_self, g0, bs, ys2)
                if bs[-1] < SPLIT_BKT:
                    store_pair(yslA, g0, bs, y162)
                else:
                    # whole group is in the B half (SPLIT_BKT is even)
                    rows_last = bucket_rows(bs[-1])
                    r0 = (g0 - SPLIT_BKT) * P
                    if rows_last == P:
                        nc.sync.dma_start(
                            pair_dram(yslB[r0:r0 + P, :], len(bs)),
                            y162[:, 0:len(bs), :])
                    else:
                        for bi, b in enumerate(bs):
                            rows = bucket_rows(b)
                            rb = (b - SPLIT_BKT) * P
                            nc.sync.dma_start(yslB[rb:rb + rows, :],
                                              y162[0:rows, bi, :])

            def gin_layer(lyr, srcA16, srcB16, self_dram, W1, W2, b1, b2,
                          gm, bt, st_in, st_out, out_dram, export_h1,
                          agA, agB):
                z2T = bigpool.tile([P, 2, ncols], BF16, tag=f"z2T_{lyr}")
                sums = spool.tile([P, 2 * ngrp], F32, tag=f"sums{lyr}")
                sumq = spool.tile([P, 2 * ngrp], F32, tag=f"sumq{lyr}")

                # AllGather half A (overlaps producer tail compute)
                nc.gpsimd.collective_compute(
                    "AllGather", OP.bypass, replica_groups=rg,
                    ins=[agA[0].opt()], outs=[agA[1].opt()])

                liveA = {}
                liveB = {}

                def issue_call(gi2, h):
                    col, nbc = call_segs[gi2][h]
                    if nbc == 0:
                        return
                    gbase = call_segs[gi2][h][0]
                    src = srcA16 if h == 0 else srcB16
                    size = cfg.tbl_a if h == 0 else cfg.tbl_b
                    gt = liveA[gi2] if h == 0 else liveB[gi2]
                    nc.gpsimd.dma_gather(
                        gt[:, col - gbase:col - gbase + nbc, :],
                        src[0:size, :],
                        idx_sb[:, col * 8:(col + nbc) * 8],
                        nbc * P, nbc * P, D, single_packet=False,
                        queue_num=next_q(nbc))

                def issue_A(gi2):
                    liveA[gi2] = gpoolA.tile([P, nbAmax, D], BF16, tag="gA",
                                             name=f"gtA{lyr}_{gi2}")
                    issue_call(gi2, 0)

                def issue_B(gi2):
                    liveB[gi2] = gpoolB.tile([P, nbBmax, D], BF16, tag="gB",
                                             name=f"gtB{lyr}_{gi2}")
                    issue_call(gi2, 1)

                for gi2 in range(min(PRO, ngrp)):
                    issue_A(gi2)
                # AllGather half B (first A-gathers already queued ahead)
                nc.gpsimd.collective_compute(
                    "AllGather", OP.bypass, replica_groups=rg,
                    ins=[agB[0].opt()], outs=[agB[1].opt()])
                if PRO < ngrp:
                    issue_A(PRO)
                issue_B(0)

                for gi, bs in groups():
                    g0 = bs[0]
                    if gi + PRO + 1 < ngrp:
                        issue_A(gi + PRO + 1)
                    if gi + 1 < ngrp:
                        issue_B(gi + 1)
                    gtA = liveA.pop(gi)
                    gtB = liveB.pop(gi)
                    xr2 = load_pair_rows(self_dram, g0, bs, "xrows")
                    p0t = wpool.tile([P, 2, D], BF16, tag="p0t")
                    for bi, b in enumerate(bs):
                        segs = bucket_segs[b]
                        total = segs[0][1] + segs[1][1]
                        p0 = psA.tile([P, D], F32, tag="aggps")
                        done = 0
                        for hseg, (col0, nbseg) in enumerate(segs):
                            gt = gtA if hseg == 0 else gtB
                            gbase = call_segs[gi][hseg][0]
                            for i0 in range(0, nbseg, 4):
                                cnt = min(4, nbseg - i0)
                                gcol = col0 + i0
                                lcol = gcol - gbase
                                mw = wpool.tile([P, 4, D], BF16, tag="msgw")
                                o_ap = _ap3(mw[:, 0:cnt, :],
                                            [[D, cnt], [DCOM, NCOM],
                                             [2, DCOM // 2], [1, 2]])
                                i0_ap = _ap3(gt[:, lcol:lcol + cnt, :],
                                             [[D, cnt], [DCOM, NCOM],
                                              [2, DCOM // 2], [1, 2]])
                                w_base = w4_sb[:, gcol * 2 * NCOM:
                                               (gcol + cnt) * 2 * NCOM]
                                w_ap = _ap3(w_base,
                                            [[2 * NCOM, cnt], [2, NCOM],
                                             [0, DCOM // 2], [1, 2]])
                                nc.vector.tensor_tensor(o_ap, i0_ap, w_ap,
                                                        OP.mult)
                                oh4 = wpool.tile([P, 4, P], BF16,
                                                 tag="onehot")
                                dcols = dst_sb[:, gcol * 2:(gcol + cnt) * 2]
                                nc.vector.tensor_tensor(
                                    _ap3(oh4[:, 0:cnt, :],
                                         [[P, cnt], [2, P // 2], [1, 2]]),
                                    _ap3(sb["iota16"][:],
                                         [[0, cnt], [2, P // 2], [1, 2]]),
                                    _ap3(dcols,
                                         [[2, cnt], [0, P // 2], [1, 2]]),
                                    OP.is_equal)
                                for r in range(cnt):
                                    nc.tensor.matmul(
                                        p0[:], oh4[:, r, :], mw[:, r, :],
                                        start=(done == 0),
                                        stop=(done == total - 1))
                                    done += 1
                        # self-add + transpose
                        p0sb = wpool.tile([P, D], F32, tag="p0sb")
                        nc.vector.tensor_tensor(p0sb[:], p0[:],
                                                xr2[:, bi, :], OP.add)
                        trp = psT.tile([P, D], F32, tag="tr")
                        for ch in range(2):
                            nc.tensor.transpose(
                                trp[:, ch * P:(ch + 1) * P],
                                p0sb[:, ch * P:(ch + 1) * P], sb["ident"][:])
                        ci = (b - g0) * P
                        for ch in range(2):
                            nc.scalar.copy(p0t[:, ch, ci:ci + P],
                                           trp[:, ch * P:(ch + 1) * P])
                    nct = len(bs) * P
                    r1 = wpool.tile([P, 2, D], BF16, tag="relu1")
                    if W1 is None:
                        # layer 1: W1 already folded into Y; z1 = P0 + b1
                        for fh in range(2):
                            nc.scalar.activation(
                                r1[:, fh, 0:nct], p0t[:, fh, 0:nct],
                                AF.Relu, bias=b1[:, fh:fh + 1], scale=1.0)
                    else:
                        z1 = ps1.tile([P, 2 * D], F32, tag="z1ps")
                        for fh in range(2):
                            for ch in range(2):
                                nc.tensor.matmul(
                                    z1[:, fh * D:fh * D + nct],
                                    W1[:, ch * D + fh * P:
                                       ch * D + (fh + 1) * P],
                                    p0t[:, ch, 0:nct],
                                    start=(ch == 0), stop=(ch == 1))
                        for fh in range(2):
                            nc.scalar.activation(
                                r1[:, fh, 0:nct], z1[:, fh * D:fh * D + nct],
                                AF.Relu, bias=b1[:, fh:fh + 1], scale=1.0)
                    z2 = ps2.tile([P, 2 * D], F32, tag="z2ps")
                    for fh in range(2):
                        for ch in range(2):
                            nc.tensor.matmul(
                                z2[:, fh * D:fh * D + nct],
                                W2[:, ch * D + fh * P:ch * D + (fh + 1) * P],
                                r1[:, ch, 0:nct],
                                start=(ch == 0), stop=(ch == 1))
                    real = min(cfg.npc - g0 * P, nct)
                    for fh in range(2):
                        nc.scalar.activation(
                            z2T[:, fh, g0 * P:g0 * P + nct],
                            z2[:, fh * D:fh * D + nct],
                            AF.Identity, bias=b2[:, fh:fh + 1])
                        # stats over real (non-pad) node columns only
                        nc.vector.tensor_reduce(
                            sums[:, fh * ngrp + gi:fh * ngrp + gi + 1],
                            z2T[:, fh, g0 * P:g0 * P + real], AX.X, OP.add)
                        sq = wpool.tile([P, D], F32, tag="sqtmp")
                        nc.scalar.square(sq[:, 0:real],
                                         z2T[:, fh, g0 * P:g0 * P + real])
                        nc.vector.tensor_reduce(
                            sumq[:, fh * ngrp + gi:fh * ngrp + gi + 1],
                            sq[:, 0:real], AX.X, OP.add)

                # ---- global stats
                pack = spool.tile([P, 4], F32, tag=f"pack{lyr}")
                for fh in range(2):
                    nc.vector.tensor_reduce(
                        pack[:, fh:fh + 1],
                        sums[:, fh * ngrp:(fh + 1) * ngrp], AX.X, OP.add)
                    nc.vector.tensor_reduce(
                        pack[:, 2 + fh:3 + fh],
                        sumq[:, fh * ngrp:(fh + 1) * ngrp], AX.X, OP.add)
                nc.sync.dma_start(st_in[:, :], pack[:])
                nc.gpsimd.collective_compute(
                    "AllReduce", OP.add, replica_groups=rg,
                    ins=[st_in.opt()], outs=[st_out.opt()])
                gl = spool.tile([P, 4], F32, tag=f"gl{lyr}")
                nc.sync.dma_start(gl[:], st_out[:, :])
                invn = 1.0 / cfg.n_nodes
                mean = spool.tile([P, 2], F32, tag=f"mean{lyr}")
                nc.vector.tensor_scalar(mean[:], gl[:, 0:2], invn, None, OP.mult)
                es2_ = spool.tile([P, 2], F32, tag=f"es2{lyr}")
                nc.vector.tensor_scalar(es2_[:], gl[:, 2:4], invn, None, OP.mult)
                msq = spool.tile([P, 2], F32, tag=f"msq{lyr}")
                nc.vector.tensor_tensor(msq[:], mean[:], mean[:], OP.mult)
                var = spool.tile([P, 2], F32, tag=f"var{lyr}")
                nc.vector.tensor_tensor(var[:], es2_[:], msq[:], OP.subtract)
                vpe = spool.tile([P, 2], F32, tag=f"vpe{lyr}")
                nc.vector.tensor_scalar(vpe[:], var[:], BN_EPS, None, OP.add)
                std = spool.tile([P, 2], F32, tag=f"std{lyr}")
                nc.scalar.activation(std[:], vpe[:], AF.Sqrt)
                rstd = spool.tile([P, 2], F32, tag=f"rstd{lyr}")
                nc.vector.reciprocal(rstd[:], std[:])
                scl = spool.tile([P, 2], F32, tag=f"scl{lyr}")
                nc.vector.tensor_tensor(scl[:], gm[:], rstd[:], OP.mult)
                mscl = spool.tile([P, 2], F32, tag=f"mscl{lyr}")
                nc.vector.tensor_tensor(mscl[:], mean[:], scl[:], OP.mult)
                bia = spool.tile([P, 2], F32, tag=f"bia{lyr}")
                nc.vector.tensor_tensor(bia[:], bt[:], mscl[:], OP.subtract)

                # ---- BN apply + transpose back + store (pair-staged)
                for gi, bs in groups():
                    g0 = bs[0]
                    hr2 = rpool.tile([P, 2, D], F32, tag="st32")
                    h162 = (rpool.tile([P, 2, D], BF16, tag="st16",
                                       name=f"h16r_{gi}")
                            if export_h1 else None)
                    for bi, b in enumerate(bs):
                        ht = wpool.tile([P, D], F32, tag="htmp")
                        for fh in range(2):
                            nc.scalar.activation(
                                ht[:, fh * P:(fh + 1) * P],
                                z2T[:, fh, b * P:(b + 1) * P],
                                AF.Relu, bias=bia[:, fh:fh + 1],
                                scale=scl[:, fh:fh + 1])
                        trp = psT.tile([P, D], F32, tag="tr")
                        for fh in range(2):
                            nc.tensor.transpose(
                                trp[:, fh * P:(fh + 1) * P],
                                ht[:, fh * P:(fh + 1) * P], sb["ident"][:])
                        nc.vector.tensor_copy(hr2[:, bi, :], trp[:])
                        if export_h1:
                            nc.scalar.copy(h162[:, bi, :], trp[:])
                    store_pair(out_dram, g0, bs, hr2)
                    if export_h1:
                        store_pair(h1_self, g0, bs, hr2)
                        if bs[-1] < SPLIT_BKT:
                            store_pair(hslA, g0, bs, h162)
                        else:
                            rows_last = bucket_rows(bs[-1])
                            r0 = (g0 - SPLIT_BKT) * P
                            if rows_last == P:
                                nc.sync.dma_start(
                                    pair_dram(hslB[r0:r0 + P, :], len(bs)),
                                    h162[:, 0:len(bs), :])
                            else:
                                for bi, b in enumerate(bs):
                                    rows = bucket_rows(b)
                                    rb = (b - SPLIT_BKT) * P
                                    nc.sync.dma_start(
                                        hslB[rb:rb + rows, :],
                                        h162[0:rows, bi, :])

            gin_layer(1, yfA, yfB, y_self, None, sb["W2r_l1"],
                      sb["b1c_l1"], sb["b2c_l1"], sb["gm1"], sb["btc1"],
                      st_in1, st_out1, out1_o, export_h1=True,
                      agA=(yslA, yfA), agB=(yslB, yfB))
            gin_layer(2, hfA, hfB, h1_self, sb["W1r_l2"], sb["W2r_l2"],
                      sb["b1c_l2"], sb["b2c_l2"], sb["gm2"], sb["btc2"],
                      st_in2, st_out2, out2_o, export_h1=False,
                      agA=(hslA, hfA), agB=(hslB, hfB))

    nc.compile()
    return nc


# ----------------------------------------------------------------------------
# Entry point
# ----------------------------------------------------------------------------

_NC_CACHE = {}
KERNEL_TRACE = False
LAST_RESULTS = None


def kernel(**inputs):
    global LAST_RESULTS
    cfg = CFG
    x = np.asarray(inputs["x"], np.float32)
    meta, idx_ts, dst_ts, w4_ts = plan(
        np.asarray(inputs["edge_index"]), inputs["ew"], cfg)
    shared = prep_weights(inputs, cfg)

    key = ("k", tuple(map(tuple, meta["nb_seg"])))
    if key not in _NC_CACHE:
        _NC_CACHE[key] = build_nc(meta, cfg)
    nc = _NC_CACHE[key]

    in_maps = []
    for c in range(N_CORES):
        m = dict(shared)
        m["x_self"] = x[c * cfg.npc:(c + 1) * cfg.npc]
        m["idx_t"] = idx_ts[c]
        m["dst_t"] = dst_ts[c]
        m["w4_t"] = w4_ts[c]
        in_maps.append(m)

    kw = {}
    if KERNEL_TRACE:
        kw = dict(trace=True, trace_cores=[0], stitch_traces=False)
    res = run_bass_kernel_spmd(nc, in_maps, core_ids=list(range(N_CORES)), **kw)
    LAST_RESULTS = res

    enc = np.concatenate([res.results[c]["enc_o"] for c in range(N_CORES)])
    out1 = np.concatenate([res.results[c]["out1_o"] for c in range(N_CORES)])
    out2 = np.concatenate([res.results[c]["out2_o"] for c in range(N_CORES)])
    return (enc, out1, out2)


# revision 47
# speedup vs baseline: 1.0240x; 1.0240x over previous
"""ComGNNBank Trainium2 kernel: 2-layer community-GIN bank + input encoder.

Strategy (8 NeuronCores, SPMD):
  - Nodes sharded by dst: core c owns rows [c*6250, (c+1)*6250).
  - Edges bucketed by dst into 128-node buckets; per bucket a weighted
    scatter-add is done on the tensor engine: one-hot(dst) matmul with
    PSUM accumulation over 128-edge blocks.
  - Gather of source-node features via SWDGE dma_gather (4 SWDGE queues,
    greedy load-balanced; one call per (2-bucket group, table half))
    from bf16 replicas of the feature table (x / h1). The replica is
    split in two halves (buckets 0..SPLIT_BKT-1 / rest) AllGather-ed
    separately so the first half's transfer overlaps producer compute
    and the first gathers overlap the second transfer.
  - GIN MLP runs feature-major (transposed, bf16) so BatchNorm+ReLU
    fuse into one per-partition scalar-engine op.
  - BN statistics: per-group partial sums inline, tiny AllReduce.
"""

import math

import ml_dtypes
import numpy as np

import concourse.bacc as bacc
import concourse.bass as bass
import concourse.mybir as mybir
import concourse.tile as tile
from concourse.bass_utils import run_bass_kernel_spmd

F32 = mybir.dt.float32
BF16 = mybir.dt.bfloat16
I16 = mybir.dt.int16
AF = mybir.ActivationFunctionType
OP = mybir.AluOpType
AX = mybir.AxisListType

P = 128
D = 256            # feature dim (in and emb)
NCOM = 4
DCOM = 64
N_CORES = 8
BN_EPS = 1e-5
IDX_LIMIT = 32768  # int16 gather index limit
SPLIT_BKT = 30     # buckets 0..29 -> table A (AllGather chunk 1)
PRO = 3            # A-gather prefetch depth (groups)
N_SWDGE_Q = 4


class _Cfg:
    def __init__(self, n_nodes):
        assert n_nodes % N_CORES == 0
        self.n_nodes = n_nodes
        self.npc = n_nodes // N_CORES
        self.nbkt = math.ceil(self.npc / P)
        self.ncols = self.nbkt * P          # padded per-core node slots
        self.last_rows = self.npc - (self.nbkt - 1) * P
        self.rows_a = min(SPLIT_BKT * P, self.npc)
        self.rows_b = self.npc - self.rows_a
        self.tbl_a = N_CORES * self.rows_a
        self.tbl_b = N_CORES * self.rows_b
        assert self.tbl_a < IDX_LIMIT and self.tbl_b < IDX_LIMIT

CFG = _Cfg(50000)


# ----------------------------------------------------------------------------
# Host planner
# ----------------------------------------------------------------------------

def blk_layout(nb_seg, nbkt):
    """Block-column layout: per 2-bucket group g the blocks are laid out
    [A(b0) | A(b1) | B(b0) | B(b1)] so one gather call covers each
    (group, half). Returns per-bucket segments, per-group call ranges,
    and the total block count."""
    blk = 0
    bucket_segs = [[] for _ in range(nbkt)]
    call_segs = []
    for g0 in range(0, nbkt, 2):
        bs = [b for b in (g0, g0 + 1) if b < nbkt]
        entry = []
        for h in range(2):
            col0 = blk
            for b in bs:
                nb = int(nb_seg[b, h])
                bucket_segs[b].append((blk, nb))
                blk += nb
            entry.append((col0, blk - col0))
        call_segs.append(entry)
    return bucket_segs, call_segs, blk


def _wrap_idx(seg_idx):
    """int16 index list (len%128==0) -> [128, len//16] wrapped layout."""
    w = seg_idx.reshape(-1, 16).T.astype(np.int16)  # [16, n//16]
    return np.tile(w, (8, 1))


def plan(edge_index, ew, cfg=CFG):
    """Bucket/pad edges; returns per-core input arrays + shared structure."""
    src = np.asarray(edge_index[0], np.int64)
    dst = np.asarray(edge_index[1], np.int64)
    ew = np.asarray(ew, np.float32)
    core = dst // cfg.npc
    ldst = dst - core * cfg.npc
    bkt = ldst // P
    off = ldst % P
    sc = src // cfg.npc
    sr = src - sc * cfg.npc
    half = (sr >= cfg.rows_a).astype(np.int64)
    ridx = np.where(half == 0, sc * cfg.rows_a + sr,
                    sc * cfg.rows_b + (sr - cfg.rows_a))

    # group edges by (core, bucket, half)
    key = (core * cfg.nbkt + bkt) * 2 + half
    order = np.argsort(key, kind="stable")
    key_s = key[order]
    nkeys = N_CORES * cfg.nbkt * 2
    starts = np.searchsorted(key_s, np.arange(nkeys + 1))
    counts = np.diff(starts).reshape(N_CORES, cfg.nbkt, 2)
    nb_seg = np.ceil(counts / P).astype(np.int64).max(axis=0)  # [nbkt, 2]

    bucket_segs, call_segs, nb_tot = blk_layout(nb_seg, cfg.nbkt)

    ridx_s = ridx[order]
    off_s = off[order]
    ew_s = ew[:, order]

    idx_ts, dst_ts, w4_ts = [], [], []
    for c in range(N_CORES):
        idx_all = np.zeros(nb_tot * P, np.int64)
        dst_all = np.full(nb_tot * P, -1.0, np.float32)
        w4_all = np.zeros((nb_tot * P, NCOM), np.float32)
        for b in range(cfg.nbkt):
            for h in range(2):
                col, nb = bucket_segs[b][h]
                if nb == 0:
                    continue
                g = (c * cfg.nbkt + b) * 2 + h
                s0, s1 = starts[g], starts[g + 1]
                n = s1 - s0
                a0 = col * P
                idx_all[a0:a0 + n] = ridx_s[s0:s1]
                dst_all[a0:a0 + n] = off_s[s0:s1]
                w4_all[a0:a0 + n] = ew_s[:, s0:s1].T
        idx_t = np.zeros((P, nb_tot * 8), np.int16)
        for b in range(cfg.nbkt):
            for h in range(2):
                col, nb = bucket_segs[b][h]
                if nb == 0:
                    continue
                seg = idx_all[col * P:(col + nb) * P]
                idx_t[:, col * 8:(col + nb) * 8] = _wrap_idx(seg)
        dst_t = np.repeat(dst_all.reshape(nb_tot, P).T, 2,
                          axis=1).astype(ml_dtypes.bfloat16)
        w4_t = (
            np.repeat(w4_all, 2, axis=1)
            .reshape(nb_tot, P, 2 * NCOM)
            .transpose(1, 0, 2)
            .reshape(P, nb_tot * 2 * NCOM)
            .astype(ml_dtypes.bfloat16)
        )
        idx_ts.append(idx_t)
        dst_ts.append(dst_t)
        w4_ts.append(w4_t)

    meta = {"nb_seg": nb_seg, "nb_tot": nb_tot}
    return meta, idx_ts, dst_ts, w4_ts


def prep_weights(inp, cfg=CFG):
    """Shared (replicated) parameter tensors in kernel layout."""
    def cat_w1(w):  # [4, ci, 64] -> [ci_tot=256 blockwise, 256]
        ci = w.shape[1]
        if ci == D:  # layer-1 first mm: dense columns concat
            return np.concatenate([w[k] for k in range(NCOM)], axis=1)
        out = np.zeros((D, D), np.float32)  # block-diag for 64-in mats
        for k in range(NCOM):
            out[k * DCOM:(k + 1) * DCOM, k * DCOM:(k + 1) * DCOM] = w[k]
        return out

    def reorder(w):  # [256, 256] -> [128, 512]: [c%128, (c//128)*256 + f]
        return np.concatenate([w[0:P, :], w[P:2 * P, :]],
                              axis=1).astype(ml_dtypes.bfloat16)

    def halves(v):  # [256] -> [128, 2]
        return np.stack([v[0:P], v[P:2 * P]], axis=1).astype(np.float32)

    w1l1 = cat_w1(np.asarray(inp["W1_0"], np.float32))
    w2l1 = cat_w1(np.asarray(inp["W2_0"], np.float32))
    w1l2 = cat_w1(np.asarray(inp["W1_1"], np.float32))
    w2l2 = cat_w1(np.asarray(inp["W2_1"], np.float32))

    out = {
        "W1r_l1": reorder(w1l1),
        "W2r_l1": reorder(w2l1),
        "W1r_l2": reorder(w1l2),
        "W2r_l2": reorder(w2l2),
        "encWr": reorder(np.asarray(inp["encW"], np.float32)),
        "b1c_l1": halves(np.asarray(inp["b1_0"], np.float32).reshape(-1)),
        "b2c_l1": halves(np.asarray(inp["b2_0"], np.float32).reshape(-1)),
        "b1c_l2": halves(np.asarray(inp["b1_1"], np.float32).reshape(-1)),
        "b2c_l2": halves(np.asarray(inp["b2_1"], np.float32).reshape(-1)),
        "gm1": halves(np.asarray(inp["g0"], np.float32).reshape(-1)),
        "btc1": halves(np.asarray(inp["bt0"], np.float32).reshape(-1)),
        "gm2": halves(np.asarray(inp["g1"], np.float32).reshape(-1)),
        "btc2": halves(np.asarray(inp["bt1"], np.float32).reshape(-1)),
        "encb_row": np.asarray(inp["encb"], np.float32).reshape(1, D)
        .astype(ml_dtypes.bfloat16),
        "ones1": np.ones((1, P), ml_dtypes.bfloat16),
        "iota16": np.broadcast_to(
            np.arange(P, dtype=np.float32), (P, P)
        ).astype(ml_dtypes.bfloat16).copy(),
        "ident": np.eye(P, dtype=np.float32),
    }
    return out


# ----------------------------------------------------------------------------
# Kernel builder
# ----------------------------------------------------------------------------

def _ap3(t_ap, ap_list):
    """Rebuild an AP with an explicit free-dim pattern list."""
    return bass.AP(t_ap.tensor, t_ap.offset, [list(t_ap.ap[0])] + ap_list)


def build_nc(meta, cfg=CFG):
    nb_seg = meta["nb_seg"]
    nb_tot = meta["nb_tot"]
    nbkt = cfg.nbkt
    ncols = cfg.ncols
    bucket_segs, call_segs, nb_tot2 = blk_layout(nb_seg, nbkt)
    assert nb_tot2 == nb_tot
    ngrp = len(call_segs)
    nbAmax = max(e[0][1] for e in call_segs)
    nbBmax = max(e[1][1] for e in call_segs)
    rg = [list(range(N_CORES))]

    nc = bacc.Bacc("TRN2", target_bir_lowering=False, debug=False,
                   num_devices=N_CORES, num_swdge_queues=N_SWDGE_Q)
    qload = [0] * N_SWDGE_Q

    def next_q(nblocks):
        q = qload.index(min(qload))
        qload[q] += nblocks
        return q

    # inputs
    x_self = nc.dram_tensor("x_self", [cfg.npc, D], F32, kind="ExternalInput")
    idx_d = nc.dram_tensor("idx_t", [P, nb_tot * 8], I16, kind="ExternalInput")
    dst_d = nc.dram_tensor("dst_t", [P, nb_tot * 2], BF16,
                           kind="ExternalInput")
    w4_d = nc.dram_tensor("w4_t", [P, nb_tot * 2 * NCOM], BF16,
                          kind="ExternalInput")
    wd = {}
    for nm, shape, dt in [
        ("W1r_l1", [P, 2 * D], BF16), ("W2r_l1", [P, 2 * D], BF16),
        ("W1r_l2", [P, 2 * D], BF16), ("W2r_l2", [P, 2 * D], BF16),
        ("encWr", [P, 2 * D], BF16),
        ("b1c_l1", [P, 2], F32), ("b2c_l1", [P, 2], F32),
        ("b1c_l2", [P, 2], F32), ("b2c_l2", [P, 2], F32),
        ("gm1", [P, 2], F32), ("btc1", [P, 2], F32),
        ("gm2", [P, 2], F32), ("btc2", [P, 2], F32),
        ("encb_row", [1, D], BF16), ("ones1", [1, P], BF16),
        ("iota16", [P, P], BF16), ("ident", [P, P], F32),
    ]:
        wd[nm] = nc.dram_tensor(nm, shape, dt, kind="ExternalInput")

    # outputs
    enc_o = nc.dram_tensor("enc_o", [cfg.npc, D], F32, kind="ExternalOutput")
    out1_o = nc.dram_tensor("out1_o", [cfg.npc, D], F32, kind="ExternalOutput")
    out2_o = nc.dram_tensor("out2_o", [cfg.npc, D], F32, kind="ExternalOutput")

    with tile.TileContext(nc) as tc:
        with (
            tc.tile_pool(name="const", bufs=1) as cpool,
            tc.tile_pool(name="big", bufs=1) as bigpool,
            tc.tile_pool(name="gatA", bufs=PRO + 2) as gpoolA,
            tc.tile_pool(name="gatB", bufs=3) as gpoolB,
            tc.tile_pool(name="work", bufs=3) as wpool,
            tc.tile_pool(name="rows", bufs=3) as rpool,
            tc.tile_pool(name="stats", bufs=1) as spool,
            tc.tile_pool(name="psA", bufs=2, space="PSUM") as psA,
            tc.tile_pool(name="psT", bufs=2, space="PSUM") as psT,
            tc.tile_pool(name="ps1", bufs=2, space="PSUM") as ps1,
            tc.tile_pool(name="ps2", bufs=2, space="PSUM") as ps2,
            tc.tile_pool(name="dram", bufs=1, space="DRAM") as dpool,
        ):
            # ---- constants into SBUF
            sb = {}
            for nm in wd:
                t = cpool.tile(list(wd[nm].shape), wd[nm].dtype, tag=nm)
                nc.sync.dma_start(t[:], wd[nm].ap())
                sb[nm] = t
            idx_sb = cpool.tile([P, nb_tot * 8], I16, tag="idx")
            nc.sync.dma_start(idx_sb[:], idx_d.ap())
            dst_sb = cpool.tile([P, nb_tot * 2], BF16, tag="dst")
            nc.sync.dma_start(dst_sb[:], dst_d.ap())
            w4_sb = cpool.tile([P, nb_tot * 2 * NCOM], BF16, tag="w4")
            nc.sync.dma_start(w4_sb[:], w4_d.ap())

            # ---- DRAM scratch
            yslA = dpool.tile([cfg.rows_a, D], BF16, tag="yslA")
            yslB = dpool.tile([cfg.rows_b, D], BF16, tag="yslB")
            yfA = dpool.tile([cfg.tbl_a, D], BF16, tag="yfA",
                             addr_space="Shared")
            yfB = dpool.tile([cfg.tbl_b, D], BF16, tag="yfB",
                             addr_space="Shared")
            y_self = dpool.tile([cfg.npc, D], F32, tag="yself")
            hslA = dpool.tile([cfg.rows_a, D], BF16, tag="hslA")
            hslB = dpool.tile([cfg.rows_b, D], BF16, tag="hslB")
            hfA = dpool.tile([cfg.tbl_a, D], BF16, tag="hfA",
                             addr_space="Shared")
            hfB = dpool.tile([cfg.tbl_b, D], BF16, tag="hfB",
                             addr_space="Shared")
            h1_self = dpool.tile([cfg.npc, D], F32, tag="h1self")
            st_in1 = dpool.tile([P, 4], F32, tag="sti1")
            st_out1 = dpool.tile([P, 4], F32, tag="sto1", addr_space="Shared")
            st_in2 = dpool.tile([P, 4], F32, tag="sti2")
            st_out2 = dpool.tile([P, 4], F32, tag="sto2", addr_space="Shared")

            def bucket_rows(b):
                return cfg.last_rows if b == nbkt - 1 else P

            def groups():
                for gi, g0 in enumerate(range(0, nbkt, 2)):
                    yield gi, [b for b in (g0, g0 + 1) if b < nbkt]

            def pair_dram(dram_slice, nb_b):
                """AP writing [P, nb_b, D] sbuf tile to nb_b*P dram rows."""
                return _ap3(dram_slice, [[P * D, nb_b], [1, D]])

            def load_pair_rows(src_dram, g0, bs, tag):
                """One DMA for a group's self rows; zero-pads last bucket."""
                t = rpool.tile([P, 2, D], F32, tag=tag)
                last = bs[-1] == nbkt - 1
                rows_last = bucket_rows(bs[-1])
                if last:
                    if len(bs) == 2:
                        nc.sync.dma_start(
                            t[:, 0, :], src_dram[g0 * P:(g0 + 1) * P, :])
                        nc.vector.memset(
                            t[(rows_last // 32) * 32:P, 1, :], 0.0)
                        nc.sync.dma_start(
                            t[0:rows_last, 1, :],
                            src_dram[(g0 + 1) * P:(g0 + 1) * P + rows_last, :])
                    else:
                        nc.vector.memset(
                            t[(rows_last // 32) * 32:P, 0, :], 0.0)
                        nc.sync.dma_start(
                            t[0:rows_last, 0, :],
                            src_dram[g0 * P:g0 * P + rows_last, :])
                else:
                    nc.sync.dma_start(
                        t[:, 0:len(bs), :],
                        pair_dram(src_dram[g0 * P:g0 * P + P, :], len(bs)))
                return t

            def store_pair(dram, g0, bs, t, fh_dim=False):
                """Store [P, len(bs), D] tile to dram rows g0*P..; handles
                the short last bucket."""
                nb_b = len(bs)
                rows_last = bucket_rows(bs[-1])
                if rows_last == P:
                    nc.sync.dma_start(
                        pair_dram(dram[g0 * P:g0 * P + P, :], nb_b),
                        t[:, 0:nb_b, :])
                else:
                    for bi, b in enumerate(bs):
                        rows = bucket_rows(b)
                        nc.sync.dma_start(
                            dram[b * P:b * P + rows, :], t[0:rows, bi, :])

            # ---- phase 0: per bucket compute enc = x@encW + encb and
            #      Y = x@W1cat; export Y f32 (self) + bf16 (gather halves).
            for gi, bs in groups():
                g0 = bs[0]
                xr2 = load_pair_rows(x_self.ap(), g0, bs, "xrows")
                es2 = rpool.tile([P, 2, D], F32, tag="st32")
                ys2 = rpool.tile([P, 2, D], F32, tag="st32")
                y162 = rpool.tile([P, 2, D], BF16, tag="st16")
                for bi, b in enumerate(bs):
                    trp = psT.tile([P, D], F32, tag="tr")
                    for ch in range(2):
                        nc.tensor.transpose(
                            trp[:, ch * P:(ch + 1) * P],
                            xr2[:, bi, ch * P:(ch + 1) * P], sb["ident"][:])
                    xT = wpool.tile([P, D], BF16, tag="encxT")
                    for ch in range(2):
                        nc.scalar.copy(xT[:, ch * P:(ch + 1) * P],
                                       trp[:, ch * P:(ch + 1) * P])
                    ep = psA.tile([P, D], F32, tag="aggps")
                    for ch in range(2):
                        nc.tensor.matmul(
                            ep[:], xT[:, ch * P:(ch + 1) * P],
                            sb["encWr"][:, ch * D:(ch + 1) * D],
                            start=(ch == 0), stop=False)
                    nc.tensor.matmul(
                        ep[:], sb["ones1"][:], sb["encb_row"][:],
                        start=False, stop=True)
                    nc.vector.tensor_copy(es2[:, bi, :], ep[:])
                    yp = ps1.tile([P, 2 * D], F32, tag="z1ps")
                    for ch in range(2):
                        nc.tensor.matmul(
                            yp[:, 0:D], xT[:, ch * P:(ch + 1) * P],
                            sb["W1r_l1"][:, ch * D:(ch + 1) * D],
                            start=(ch == 0), stop=(ch == 1))
                    nc.vector.tensor_copy(ys2[:, bi, :], yp[:, 0:D])
                    nc.scalar.copy(y162[:, bi, :], yp[:, 0:D])
                store_pair(enc_o, g0, bs, es2)
                store_pair(y_self, g0, bs, ys2)
                if bs[-1] < SPLIT_BKT:
                    store_pair(yslA, g0, bs, y162)
                else:
                    # whole group is in the B half (SPLIT_BKT is even)
                    rows_last = bucket_rows(bs[-1])
                    r0 = (g0 - SPLIT_BKT) * P
                    if rows_last == P:
                        nc.sync.dma_start(
                            pair_dram(yslB[r0:r0 + P, :], len(bs)),
                            y162[:, 0:len(bs), :])
                    else:
                        for bi, b in enumerate(bs):
                            rows = bucket_rows(b)
                            rb = (b - SPLIT_BKT) * P
                            nc.sync.dma_start(yslB[rb:rb + rows, :],
                                              y162[0:rows, bi, :])

            def gin_layer(lyr, srcA16, srcB16, self_dram, W1, W2, b1, b2,
                          gm, bt, st_in, st_out, out_dram, export_h1,
                          agA, agB):
                z2T = bigpool.tile([P, 2, ncols], BF16, tag=f"z2T_{lyr}")
                sums = spool.tile([P, 2 * ngrp], F32, tag=f"sums{lyr}")
                sumq = spool.tile([P, 2 * ngrp], F32, tag=f"sumq{lyr}")

                # AllGather half A (overlaps producer tail compute)
                nc.gpsimd.collective_compute(
                    "AllGather", OP.bypass, replica_groups=rg,
                    ins=[agA[0].opt()], outs=[agA[1].opt()])

                liveA = {}
                liveB = {}

                def issue_call(gi2, h):
                    col, nbc = call_segs[gi2][h]
                    if nbc == 0:
                        return
                    gbase = call_segs[gi2][h][0]
                    src = srcA16 if h == 0 else srcB16
                    size = cfg.tbl_a if h == 0 else cfg.tbl_b
                    gt = liveA[gi2] if h == 0 else liveB[gi2]
                    nc.gpsimd.dma_gather(
                        gt[:, col - gbase:col - gbase + nbc, :],
                        src[0:size, :],
                        idx_sb[:, col * 8:(col + nbc) * 8],
                        nbc * P, nbc * P, D, single_packet=False,
                        queue_num=next_q(nbc))

                def issue_A(gi2):
                    liveA[gi2] = gpoolA.tile([P, nbAmax, D], BF16, tag="gA",
                                             name=f"gtA{lyr}_{gi2}")
                    issue_call(gi2, 0)

                def issue_B(gi2):
                    liveB[gi2] = gpoolB.tile([P, nbBmax, D], BF16, tag="gB",
                                             name=f"gtB{lyr}_{gi2}")
                    issue_call(gi2, 1)

                for gi2 in range(min(PRO, ngrp)):
                    issue_A(gi2)
                # AllGather half B (first A-gathers already queued ahead)
                nc.gpsimd.collective_compute(
                    "AllGather", OP.bypass, replica_groups=rg,
                    ins=[agB[0].opt()], outs=[agB[1].opt()])
                if PRO < ngrp:
                    issue_A(PRO)
                issue_B(0)

                for gi, bs in groups():
                    g0 = bs[0]
                    if gi + PRO + 1 < ngrp:
                        issue_A(gi + PRO + 1)
                    if gi + 1 < ngrp:
                        issue_B(gi + 1)
                    gtA = liveA.pop(gi)
                    gtB = liveB.pop(gi)
                    xr2 = load_pair_rows(self_dram, g0, bs, "xrows")
                    p0t = wpool.tile([P, 2, D], BF16, tag="p0t")
                    for bi, b in enumerate(bs):
                        segs = bucket_segs[b]
                        total = segs[0][1] + segs[1][1]
                        p0 = psA.tile([P, D], F32, tag="aggps")
                        done = 0
                        for hseg, (col0, nbseg) in enumerate(segs):
                            gt = gtA if hseg == 0 else gtB
                            gbase = call_segs[gi][hseg][0]
                            for i0 in range(0, nbseg, 4):
                                cnt = min(4, nbseg - i0)
                                gcol = col0 + i0
                                lcol = gcol - gbase
                                mw = wpool.tile([P, 4, D], BF16, tag="msgw", bufs=4)
                                o_ap = _ap3(mw[:, 0:cnt, :],
                                            [[D, cnt], [DCOM, NCOM],
                                             [2, DCOM // 2], [1, 2]])
                                i0_ap = _ap3(gt[:, lcol:lcol + cnt, :],
                                             [[D, cnt], [DCOM, NCOM],
                                              [2, DCOM // 2], [1, 2]])
                                w_base = w4_sb[:, gcol * 2 * NCOM:
                                               (gcol + cnt) * 2 * NCOM]
                                w_ap = _ap3(w_base,
                                            [[2 * NCOM, cnt], [2, NCOM],
                                             [0, DCOM // 2], [1, 2]])
                                nc.vector.tensor_tensor(o_ap, i0_ap, w_ap,
                                                        OP.mult)
                                oh4 = wpool.tile([P, 4, P], BF16,
                                                 tag="onehot", bufs=4)
                                dcols = dst_sb[:, gcol * 2:(gcol + cnt) * 2]
                                nc.vector.tensor_tensor(
                                    _ap3(oh4[:, 0:cnt, :],
                                         [[P, cnt], [2, P // 2], [1, 2]]),
                                    _ap3(sb["iota16"][:],
                                         [[0, cnt], [2, P // 2], [1, 2]]),
                                    _ap3(dcols,
                                         [[2, cnt], [0, P // 2], [1, 2]]),
                                    OP.is_equal)
                                for r in range(cnt):
                                    nc.tensor.matmul(
                                        p0[:], oh4[:, r, :], mw[:, r, :],
                                        start=(done == 0),
                                        stop=(done == total - 1))
                                    done += 1
                        # self-add + transpose
                        p0sb = wpool.tile([P, D], F32, tag="p0sb")
                        nc.vector.tensor_tensor(p0sb[:], p0[:],
                                                xr2[:, bi, :], OP.add)
                        trp = psT.tile([P, D], F32, tag="tr")
                        for ch in range(2):
                            nc.tensor.transpose(
                                trp[:, ch * P:(ch + 1) * P],
                                p0sb[:, ch * P:(ch + 1) * P], sb["ident"][:])
                        ci = (b - g0) * P
                        for ch in range(2):
                            nc.scalar.copy(p0t[:, ch, ci:ci + P],
                                           trp[:, ch * P:(ch + 1) * P])
                    nct = len(bs) * P
                    r1 = wpool.tile([P, 2, D], BF16, tag="relu1")
                    if W1 is None:
                        # layer 1: W1 already folded into Y; z1 = P0 + b1
                        for fh in range(2):
                            nc.scalar.activation(
                                r1[:, fh, 0:nct], p0t[:, fh, 0:nct],
                                AF.Relu, bias=b1[:, fh:fh + 1], scale=1.0)
                    else:
                        z1 = ps1.tile([P, 2 * D], F32, tag="z1ps")
                        for fh in range(2):
                            for ch in range(2):
                                nc.tensor.matmul(
                                    z1[:, fh * D:fh * D + nct],
                                    W1[:, ch * D + fh * P:
                                       ch * D + (fh + 1) * P],
                                    p0t[:, ch, 0:nct],
                                    start=(ch == 0), stop=(ch == 1))
                        for fh in range(2):
                            nc.scalar.activation(
                                r1[:, fh, 0:nct], z1[:, fh * D:fh * D + nct],
                                AF.Relu, bias=b1[:, fh:fh + 1], scale=1.0)
                    z2 = ps2.tile([P, 2 * D], F32, tag="z2ps")
                    for fh in range(2):
                        for ch in range(2):
                            nc.tensor.matmul(
                                z2[:, fh * D:fh * D + nct],
                                W2[:, ch * D + fh * P:ch * D + (fh + 1) * P],
                                r1[:, ch, 0:nct],
                                start=(ch == 0), stop=(ch == 1))
                    real = min(cfg.npc - g0 * P, nct)
                    for fh in range(2):
                        nc.scalar.activation(
                            z2T[:, fh, g0 * P:g0 * P + nct],
                            z2[:, fh * D:fh * D + nct],
                            AF.Identity, bias=b2[:, fh:fh + 1])
                        # stats over real (non-pad) node columns only
                        nc.vector.tensor_reduce(
                            sums[:, fh * ngrp + gi:fh * ngrp + gi + 1],
                            z2T[:, fh, g0 * P:g0 * P + real], AX.X, OP.add)
                        sq = wpool.tile([P, D], F32, tag="sqtmp")
                        nc.scalar.square(sq[:, 0:real],
                                         z2T[:, fh, g0 * P:g0 * P + real])
                        nc.vector.tensor_reduce(
                            sumq[:, fh * ngrp + gi:fh * ngrp + gi + 1],
                            sq[:, 0:real], AX.X, OP.add)

                # ---- global stats
                pack = spool.tile([P, 4], F32, tag=f"pack{lyr}")
                for fh in range(2):
                    nc.vector.tensor_reduce(
                        pack[:, fh:fh + 1],
                        sums[:, fh * ngrp:(fh + 1) * ngrp], AX.X, OP.add)
                    nc.vector.tensor_reduce(
                        pack[:, 2 + fh:3 + fh],
                        sumq[:, fh * ngrp:(fh + 1) * ngrp], AX.X, OP.add)
                nc.sync.dma_start(st_in[:, :], pack[:])
                nc.gpsimd.collective_compute(
                    "AllReduce", OP.add, replica_groups=rg,
                    ins=[st_in.opt()], outs=[st_out.opt()])
                gl = spool.tile([P, 4], F32, tag=f"gl{lyr}")
                nc.sync.dma_start(gl[:], st_out[:, :])
                invn = 1.0 / cfg.n_nodes
                mean = spool.tile([P, 2], F32, tag=f"mean{lyr}")
                nc.vector.tensor_scalar(mean[:], gl[:, 0:2], invn, None, OP.mult)
                es2_ = spool.tile([P, 2], F32, tag=f"es2{lyr}")
                nc.vector.tensor_scalar(es2_[:], gl[:, 2:4], invn, None, OP.mult)
                msq = spool.tile([P, 2], F32, tag=f"msq{lyr}")
                nc.vector.tensor_tensor(msq[:], mean[:], mean[:], OP.mult)
                var = spool.tile([P, 2], F32, tag=f"var{lyr}")
                nc.vector.tensor_tensor(var[:], es2_[:], msq[:], OP.subtract)
                vpe = spool.tile([P, 2], F32, tag=f"vpe{lyr}")
                nc.vector.tensor_scalar(vpe[:], var[:], BN_EPS, None, OP.add)
                std = spool.tile([P, 2], F32, tag=f"std{lyr}")
                nc.scalar.activation(std[:], vpe[:], AF.Sqrt)
                rstd = spool.tile([P, 2], F32, tag=f"rstd{lyr}")
                nc.vector.reciprocal(rstd[:], std[:])
                scl = spool.tile([P, 2], F32, tag=f"scl{lyr}")
                nc.vector.tensor_tensor(scl[:], gm[:], rstd[:], OP.mult)
                mscl = spool.tile([P, 2], F32, tag=f"mscl{lyr}")
                nc.vector.tensor_tensor(mscl[:], mean[:], scl[:], OP.mult)
                bia = spool.tile([P, 2], F32, tag=f"bia{lyr}")
                nc.vector.tensor_tensor(bia[:], bt[:], mscl[:], OP.subtract)

                # ---- BN apply + transpose back + store (pair-staged)
                for gi, bs in groups():
                    g0 = bs[0]
                    hr2 = rpool.tile([P, 2, D], F32, tag="st32")
                    h162 = (rpool.tile([P, 2, D], BF16, tag="st16",
                                       name=f"h16r_{gi}")
                            if export_h1 else None)
                    for bi, b in enumerate(bs):
                        ht = wpool.tile([P, D], F32, tag="htmp")
                        for fh in range(2):
                            nc.scalar.activation(
                                ht[:, fh * P:(fh + 1) * P],
                                z2T[:, fh, b * P:(b + 1) * P],
                                AF.Relu, bias=bia[:, fh:fh + 1],
                                scale=scl[:, fh:fh + 1])
                        trp = psT.tile([P, D], F32, tag="tr")
                        for fh in range(2):
                            nc.tensor.transpose(
                                trp[:, fh * P:(fh + 1) * P],
                                ht[:, fh * P:(fh + 1) * P], sb["ident"][:])
                        nc.vector.tensor_copy(hr2[:, bi, :], trp[:])
                        if export_h1:
                            nc.scalar.copy(h162[:, bi, :], trp[:])
                    store_pair(out_dram, g0, bs, hr2)
                    if export_h1:
                        store_pair(h1_self, g0, bs, hr2)
                        if bs[-1] < SPLIT_BKT:
                            store_pair(hslA, g0, bs, h162)
                        else:
                            rows_last = bucket_rows(bs[-1])
                            r0 = (g0 - SPLIT_BKT) * P
                            if rows_last == P:
                                nc.sync.dma_start(
                                    pair_dram(hslB[r0:r0 + P, :], len(bs)),
                                    h162[:, 0:len(bs), :])
                            else:
                                for bi, b in enumerate(bs):
                                    rows = bucket_rows(b)
                                    rb = (b - SPLIT_BKT) * P
                                    nc.sync.dma_start(
                                        hslB[rb:rb + rows, :],
                                        h162[0:rows, bi, :])

            gin_layer(1, yfA, yfB, y_self, None, sb["W2r_l1"],
                      sb["b1c_l1"], sb["b2c_l1"], sb["gm1"], sb["btc1"],
                      st_in1, st_out1, out1_o, export_h1=True,
                      agA=(yslA, yfA), agB=(yslB, yfB))
            gin_layer(2, hfA, hfB, h1_self, sb["W1r_l2"], sb["W2r_l2"],
                      sb["b1c_l2"], sb["b2c_l2"], sb["gm2"], sb["btc2"],
                      st_in2, st_out2, out2_o, export_h1=False,
                      agA=(hslA, hfA), agB=(hslB, hfB))

    nc.compile()
    return nc


# ----------------------------------------------------------------------------
# Entry point
# ----------------------------------------------------------------------------

_NC_CACHE = {}
KERNEL_TRACE = False
LAST_RESULTS = None


def kernel(**inputs):
    global LAST_RESULTS
    cfg = CFG
    x = np.asarray(inputs["x"], np.float32)
    meta, idx_ts, dst_ts, w4_ts = plan(
        np.asarray(inputs["edge_index"]), inputs["ew"], cfg)
    shared = prep_weights(inputs, cfg)

    key = ("k", tuple(map(tuple, meta["nb_seg"])))
    if key not in _NC_CACHE:
        _NC_CACHE[key] = build_nc(meta, cfg)
    nc = _NC_CACHE[key]

    in_maps = []
    for c in range(N_CORES):
        m = dict(shared)
        m["x_self"] = x[c * cfg.npc:(c + 1) * cfg.npc]
        m["idx_t"] = idx_ts[c]
        m["dst_t"] = dst_ts[c]
        m["w4_t"] = w4_ts[c]
        in_maps.append(m)

    kw = {}
    if KERNEL_TRACE:
        kw = dict(trace=True, trace_cores=[0], stitch_traces=False)
    res = run_bass_kernel_spmd(nc, in_maps, core_ids=list(range(N_CORES)), **kw)
    LAST_RESULTS = res

    enc = np.concatenate([res.results[c]["enc_o"] for c in range(N_CORES)])
    out1 = np.concatenate([res.results[c]["out1_o"] for c in range(N_CORES)])
    out2 = np.concatenate([res.results[c]["out2_o"] for c in range(N_CORES)])
    return (enc, out1, out2)
